# revision 30
# baseline (speedup 1.0000x reference)
"""MultiHeadedAttention (sparse_attention) Trainium2 Bass kernel.

Problem: B=4, S=1024, D=1024, H=16 heads (hd=64).
  q/k/v = (x @ W.T + b) -> heads; three softmax branches (scores, time-decay,
  relative-count) blended with runtime scalars l1,l2; returns (out, prob_attn).

Sharding: 8 cores = 4 batches x 2 head-groups (8 heads each). Pure SPMD:
host slices inputs per core, kernel computes its [1, 8, S, S] prob slice and
[S, 512] out columns, host reassembles.

Per-core dataflow (all matmuls fp16 operands, fp32 PSUM accumulation):
  A) projections qT/kT [o=512, s=1024] (0.125 prescaled into Wq) and
     v [s=1024, o=512], via host-transposed xT and W.T slices.
  B) head-independent branches: C = c_t*time_attn + c_r*rel_attn  [S, S] fp32.
  C) per (head, q-tile): scores psum = qT.T@kT + I.T@maskneg (mask add on PE);
     rowmax (DVE) -> exp+rowsum (ACT, from PSUM) -> prob = p*(c_s/Z) + C
     (GpSimd) -> DMA out; PE-transpose prob -> probT fp16 -> PV matmul
     out^T[hd, q] -> DMA (host transposes back).
"""
import sys
import numpy as np

sys.path.insert(0, "/opt/trn_rl_repo")

import concourse.bass as bass  # noqa: E402
import concourse.mybir as mybir  # noqa: E402
from concourse.tile import TileContext  # noqa: E402
from concourse.bass_utils import run_bass_kernel_spmd  # noqa: E402

B, S, D, H = 4, 1024, 1024, 16
HG = H // 2          # heads per core (8)
OG = HG * (D // H)   # out columns per core (512)
HD = D // H          # 64
NT = S // 128        # 8 q/k tiles
F32 = mybir.dt.float32
F16 = mybir.dt.float16
U8 = mybir.dt.uint8
MASKNEG = -30000.0


def build(nc: bass.Bass):
    # ---- DRAM IO ----
    xq = nc.declare_dram_parameter("xq", [S, S], F16, isOutput=False)   # query[b].T
    xk = nc.declare_dram_parameter("xk", [S, S], F16, isOutput=False)
    xv = nc.declare_dram_parameter("xv", [S, S], F16, isOutput=False)
    wq = nc.declare_dram_parameter("wq", [D, OG], F16, isOutput=False)  # (0.125*Wq[g]).T
    wk = nc.declare_dram_parameter("wk", [D, OG], F16, isOutput=False)
    wv = nc.declare_dram_parameter("wv", [D, OG], F16, isOutput=False)
    bqk = nc.declare_dram_parameter("bqk", [128, 8], F32, isOutput=False)  # cols 0-3 bq', 4-7 bk
    bvb = nc.declare_dram_parameter("bvb", [128, OG], F32, isOutput=False)  # bv broadcast
    rel = nc.declare_dram_parameter("rel", [S, S], F32, isOutput=False)
    tsd = nc.declare_dram_parameter("tsd", [S, S], F32, isOutput=False)
    msk = nc.declare_dram_parameter("msk", [S, S], U8, isOutput=False)
    mneg = nc.declare_dram_parameter("mneg", [S, S], F16, isOutput=False)  # mask * -30000
    lam = nc.declare_dram_parameter("lam", [128, 4], F32, isOutput=False)  # c_s, c_t, c_r
    idn16 = nc.declare_dram_parameter("idn16", [128, 128], F16, isOutput=False)
    idn32 = nc.declare_dram_parameter("idn32", [128, 128], F32, isOutput=False)
    prob_o = nc.declare_dram_parameter("prob_o", [HG, S, S], F32, isOutput=True)
    outT_o = nc.declare_dram_parameter("outT_o", [HG, HD, S], F32, isOutputTrue := True)

    with TileContext(nc) as tc:
        # ---------- persistent pools ----------
        with tc.tile_pool(name="const", bufs=1) as pc, \
             tc.tile_pool(name="proj", bufs=1) as pj, \
             tc.tile_pool(name="maskp", bufs=1) as pm, \
             tc.tile_pool(name="cpool", bufs=1) as pC:

            bqk_t = pc.tile([128, 8], F32, tag="bqk")
            bvb_t = pc.tile([128, OG], F32, tag="bvb")
            lam_t = pc.tile([128, 4], F32, tag="lam")
            i16 = pc.tile([128, 128], F16, tag="i16")
            i32 = pc.tile([128, 128], F32, tag="i32")
            nc.sync.dma_start(out=bqk_t[:], in_=bqk[:])
            nc.sync.dma_start(out=bvb_t[:], in_=bvb[:])
            nc.sync.dma_start(out=lam_t[:], in_=lam[:])
            nc.sync.dma_start(out=i16[:], in_=idn16[:])
            nc.sync.dma_start(out=i32[:], in_=idn32[:])

            qT, kT = [], []
            for c in range(4):
                qTc = pj.tile([128, S], F16, tag=f"qT{c}")
                kTc = pj.tile([128, S], F16, tag=f"kT{c}")
                qT.append(qTc)
                kT.append(kTc)
            vT = pj.tile([128, NT, OG], F16, tag="vT")  # [s%128, s//128, o]

            mneg_t = pm.tile([128, NT, S], F16, tag="mneg")
            nc.sync.dma_start(out=mneg_t[:], in_=mneg.rearrange("(n p) m -> p n m", p=128))

            C_t = pC.tile([128, NT, S], F32, tag="C")  # c_t*T + c_r*R

            # ---------- phase A pools + loads first (DMAs at t=0), then
            # phase B body (DVE/ACT/POOL) overlapping phase A matmuls (PE) ----
            _px_cm = tc.tile_pool(name="xw", bufs=1)
            _psA_cm = tc.tile_pool(name="psA", bufs=4, space="PSUM")
            px = _px_cm.__enter__(); psA = _psA_cm.__enter__()
            xq_t = px.tile([128, 8, S], F16, tag="xq")
            xk_t = px.tile([128, 8, S], F16, tag="xk")
            xv_t = px.tile([128, 8, S], F16, tag="xv")
            wq_t = px.tile([128, 8, OG], F16, tag="wq")
            wk_t = px.tile([128, 8, OG], F16, tag="wk")
            wv_t = px.tile([128, 8, OG], F16, tag="wv")
            for t, src_ in ((xq_t, xq), (xk_t, xk), (xv_t, xv),
                            (wq_t, wq), (wk_t, wk), (wv_t, wv)):
                nc.sync.dma_start(out=t[:], in_=src_.rearrange("(n p) m -> p n m", p=128))

            with tc.tile_pool(name="br", bufs=2) as pb, \
                 tc.tile_pool(name="brz", bufs=12) as pz:
                for t in range(NT):
                    rel_t = pb.tile([128, S], F32, tag="rel")
                    ts_t = pb.tile([128, S], F32, tag="ts")
                    msk_t = pb.tile([128, S], U8, tag="msk")
                    nc.sync.dma_start(out=rel_t[:], in_=rel[t * 128:(t + 1) * 128, :])
                    nc.sync.dma_start(out=ts_t[:], in_=tsd[t * 128:(t + 1) * 128, :])
                    nc.sync.dma_start(out=msk_t[:], in_=msk[t * 128:(t + 1) * 128, :])

                    # time branch: f = exp(exp(-|ts|) + maskneg), Z1 = rowsum
                    tmp1 = pb.tile([128, S], F32, tag="tmp1")
                    tmp2 = pb.tile([128, S], F32, tag="tmp2")
                    f_t = pb.tile([128, S], F32, tag="f")
                    nc.vector.tensor_scalar(
                        tmp1.bitcast(mybir.dt.uint32)[:], ts_t.bitcast(mybir.dt.uint32)[:],
                        0x7FFFFFFF, None, mybir.AluOpType.bitwise_and)
                    nc.scalar.activation(tmp2[:], tmp1[:], mybir.ActivationFunctionType.Exp,
                                         scale=-1.0)
                    nc.gpsimd.tensor_tensor(tmp1[:], tmp2[:], mneg_t[:, t, :], mybir.AluOpType.add)
                    z1 = pz.tile([128, 1], F32, tag="z1")
                    nc.scalar.activation(f_t[:], tmp1[:], mybir.ActivationFunctionType.Exp,
                                         accum_out=z1[:])

                    # rel branch: relm = rel*mask; rel_m = relm - 1e4*(relm==0)
                    nc.gpsimd.tensor_tensor(tmp2[:], rel_t[:], msk_t[:], mybir.AluOpType.mult)
                    nc.vector.tensor_scalar(tmp1[:], tmp2[:], 0.0, -10000.0,
                                            mybir.AluOpType.is_equal, mybir.AluOpType.mult)
                    negm2 = pz.tile([128, 1], F32, tag="nm2")
                    nc.gpsimd.tensor_tensor(rel_t[:], tmp1[:], tmp2[:], mybir.AluOpType.add)
                    nc.vector.tensor_reduce(negm2[:], rel_t[:], axis=mybir.AxisListType.X,
                                            op=mybir.AluOpType.max, negate=True)
                    z2 = pz.tile([128, 1], F32, tag="z2")
                    nc.scalar.activation(tmp1[:], rel_t[:], mybir.ActivationFunctionType.Exp,
                                         bias=negm2[:], accum_out=z2[:])

                    # C = f*(c_t/Z1) + g*(c_r/Z2)
                    u1 = pz.tile([128, 1], F32, tag="u1")
                    u2 = pz.tile([128, 1], F32, tag="u2")
                    nc.vector.reciprocal(u1[:], z1[:])
                    nc.vector.reciprocal(u2[:], z2[:])
                    nc.vector.tensor_tensor(u1[:], u1[:], lam_t[:, 1:2], mybir.AluOpType.mult)
                    nc.vector.tensor_tensor(u2[:], u2[:], lam_t[:, 2:3], mybir.AluOpType.mult)
                    nc.gpsimd.tensor_scalar(tmp2[:], tmp1[:], u2[:], None, mybir.AluOpType.mult)
                    nc.vector.scalar_tensor_tensor(
                        C_t[:, t, :], f_t[:], u1[:], tmp2[:],
                        op0=mybir.AluOpType.mult, op1=mybir.AluOpType.add)

            # ---------- phase A: projections ----------
            if True:
                # qT / kT: [o 128-tile c, s 512-half] = sum_d W'[d, o].T @ xT[d, s]
                for (wt, xt, dst, bcol) in ((wq_t, xq_t, qT, 0), (wk_t, xk_t, kT, 4)):
                    for c in range(4):
                        for h2 in range(2):
                            ps = psA.tile([128, 512], F32, tag="psA")
                            for kk in range(8):
                                nc.tensor.matmul(
                                    ps[:],
                                    wt[:, kk, c * 128:(c + 1) * 128],
                                    xt[:, kk, h2 * 512:(h2 + 1) * 512],
                                    start=(kk == 0), stop=(kk == 7),
                                )
                            nc.scalar.activation(
                                dst[c][:, h2 * 512:(h2 + 1) * 512], ps[:],
                                mybir.ActivationFunctionType.Identity,
                                bias=bqk_t[:, bcol + c:bcol + c + 1], scale=1.0,
                            )
                # v: [s 128-tile r, o 512] = sum_d xT[d, s-tile].T @ W'[d, o]
                for r in range(NT):
                    ps = psA.tile([128, 512], F32, tag="psA")
                    for kk in range(8):
                        nc.tensor.matmul(
                            ps[:],
                            xv_t[:, kk, r * 128:(r + 1) * 128],
                            wv_t[:, kk, :],
                            start=(kk == 0), stop=(kk == 7),
                        )
                    nc.vector.tensor_tensor(vT[:, r, :], ps[:], bvb_t[:], mybir.AluOpType.add)

            _psA_cm.__exit__(None, None, None)
            _px_cm.__exit__(None, None, None)

            # ---------- phase C: attention ----------
            with tc.tile_pool(name="psS", bufs=2, space="PSUM") as psS, \
                 tc.tile_pool(name="psB", bufs=1, space="PSUM") as psB, \
                 tc.tile_pool(name="psV", bufs=1, space="PSUM") as psV, \
                 tc.tile_pool(name="pp", bufs=4) as pp, \
                 tc.tile_pool(name="pprob", bufs=4) as ppr, \
                 tc.tile_pool(name="pT", bufs=2) as pT, \
                 tc.tile_pool(name="pct", bufs=1) as pct, \
                 tc.tile_pool(name="pzc", bufs=16) as pzc, \
                 tc.tile_pool(name="pout", bufs=2) as po:
                # C^T (fp16) once: cast then DMA-transpose per q-tile
                CT16 = pct.tile([128, NT, S], F16, tag="CT16")
                for t in range(NT):
                    c16 = ppr.tile([128, S], F16, tag="c16")
                    if t % 2 == 0:
                        nc.vector.tensor_copy(c16[:], C_t[:, t, :])
                    else:
                        nc.scalar.copy(c16[:], C_t[:, t, :])
                    nc.sync.dma_start_transpose(CT16[:, :, t * 128:(t + 1) * 128], c16[:])
                # CV^T[o, q] = sum_k v[k, o].T @ C^T[k, q]  (once per core)
                CVT = pct.tile([128, 4, S], F32, tag="CVT")
                for c in range(4):
                    pv = psB.tile([128, S], F32, tag="big")
                    for half in range(2):
                        for kk in range(NT):
                            nc.tensor.matmul(
                                pv[:, half * 512:(half + 1) * 512],
                                vT[:, kk, c * 128:(c + 1) * 128],
                                CT16[:, kk, half * 512:(half + 1) * 512],
                                start=(kk == 0), stop=(kk == 7))
                    if c % 2 == 0:
                        nc.vector.tensor_copy(CVT[:, c, :], pv[:])
                    else:
                        nc.scalar.copy(CVT[:, c, :], pv[:])

                for h in range(HG):
                    c, off = h // 2, 64 * (h % 2)
                    pTt = pT.tile([128, NT, S], F16, tag="pT")
                    for t in range(NT):
                        ps = psS.tile([128, S], F32, tag="psS")
                        for kk in range(2):
                            nc.tensor.matmul(
                                ps[:, kk * 512:(kk + 1) * 512],
                                qT[c][off:off + 64, t * 128:(t + 1) * 128],
                                kT[c][off:off + 64, kk * 512:(kk + 1) * 512],
                                start=True, stop=False)
                        for kk in range(2):
                            nc.tensor.matmul(
                                ps[:, kk * 512:(kk + 1) * 512],
                                i16[:],
                                mneg_t[:, t, kk * 512:(kk + 1) * 512],
                                start=False, stop=True)
                        negm = pzc.tile([128, 1], F32, tag="negm")
                        nc.vector.tensor_reduce(negm[:], ps[:], axis=mybir.AxisListType.X,
                                                op=mybir.AluOpType.max, negate=True)
                        p16 = pp.tile([128, S], F16, tag="p")
                        zr = pzc.tile([128, 1], F32, tag="zr")
                        nc.scalar.activation(p16[:], ps[:], mybir.ActivationFunctionType.Exp,
                                             bias=negm[:], accum_out=zr[:])
                        u = pzc.tile([128, 1], F32, tag="u")
                        nc.vector.reciprocal(u[:], zr[:])
                        nc.vector.tensor_tensor(u[:], u[:], lam_t[:, 0:1], mybir.AluOpType.mult)
                        # p16u = p16 * u (fp16) -> DMA-transposed for PV
                        p16u = pp.tile([128, S], F16, tag="pu")
                        if (h * NT + t) % 2 == 0:
                            nc.vector.tensor_scalar(p16u[:], p16[:], u[:], None,
                                                    mybir.AluOpType.mult)
                        else:
                            nc.gpsimd.tensor_scalar(p16u[:], p16[:], u[:], None,
                                                    mybir.AluOpType.mult)
                        nc.sync.dma_start_transpose(pTt[:, :, t * 128:(t + 1) * 128], p16u[:])
                        # prob = p16u + C (output path only; off the PV critical path)
                        prob_t = ppr.tile([128, S], F32, tag="prob")
                        if (h * NT + t) % 4 == 3:
                            nc.gpsimd.tensor_tensor(prob_t[:], p16u[:], C_t[:, t, :],
                                                    mybir.AluOpType.add)
                        else:
                            nc.vector.tensor_tensor(prob_t[:], p16u[:], C_t[:, t, :],
                                                    mybir.AluOpType.add)
                        nc.sync.dma_start(out=prob_o[h, t * 128:(t + 1) * 128, :], in_=prob_t[:])

                    # PV: out^T[hd, q] = sum_k v[k, hd].T @ (u*p)^T[k, q] + CV^T
                    psv = psV.tile([64, S], F32, tag="psV")
                    for half in range(2):
                        for kk in range(NT):
                            nc.tensor.matmul(
                                psv[:, half * 512:(half + 1) * 512],
                                vT[:, kk, h * 64:h * 64 + 64],
                                pTt[:, kk, half * 512:(half + 1) * 512],
                                start=(kk == 0), stop=(kk == 7))
                    ot = po.tile([64, S], F32, tag="ot")
                    nc.vector.tensor_tensor(ot[:], psv[:], CVT[off:off + 64, c, :],
                                            mybir.AluOpType.add)
                    nc.sync.dma_start(out=outT_o[h], in_=ot[:])
    return nc


def _legalize_waits(nc, cap=1):
    """This walrus encodes at most `cap` sync-wait per instruction; hoist
    excess on_wait entries into same-engine NoOps placed just before."""
    nsplit = 0
    for fn in nc.m.functions:
        for blk in fn.blocks:
            new = []
            for inst in blk.instructions:
                si = getattr(inst, "sync_info", None)
                waits = list(si.on_wait) if si is not None and si.on_wait else []
                if len(waits) > cap:
                    extra, keep = waits[:-cap], waits[-cap:]
                    for i in range(0, len(extra), cap):
                        nop = mybir.InstNoOp(name=f"WSPLIT-{nsplit}", ins=[], outs=[])
                        nsplit += 1
                        nop.engine = inst.engine
                        nop.sync_info = mybir.SyncInfo(
                            on_wait=extra[i:i + cap], on_update=[])
                        new.append(nop)
                    si.on_wait = keep
                new.append(inst)
            blk.instructions = new
    return nsplit


_NC_CACHE = None


def _get_nc():
    global _NC_CACHE
    if _NC_CACHE is None:
        nc = build(bass.Bass())
        _legalize_waits(nc)
        _NC_CACHE = nc
    return _NC_CACHE


def _make_in_maps(query, key, value, rel, timestamp, l1, l2, mask, Wq, bq, Wk, bk, Wv, bv):
    f16 = np.float16
    l1 = np.float32(l1)
    l2 = np.float32(l2)
    c_s = (1 - l1) * (1 - l2)
    c_t = (1 - l1) * l2
    c_r = l1
    lam = np.zeros((128, 4), np.float32)
    lam[:, 0], lam[:, 1], lam[:, 2] = c_s, c_t, c_r
    idn16 = np.eye(128, dtype=f16)
    idn32 = np.eye(128, dtype=np.float32)

    in_maps = []
    for cid in range(8):
        b, g = divmod(cid, 2)
        sl = slice(g * OG, (g + 1) * OG)
        bqk2 = np.zeros((128, 8), np.float32)
        bqk2[:, 0:4] = (0.125 * bq[sl]).reshape(4, 128).T
        bqk2[:, 4:8] = bk[sl].reshape(4, 128).T
        m_u8 = mask[b].astype(np.uint8)
        in_maps.append({
            "xq": np.ascontiguousarray(query[b].T).astype(f16),
            "xk": np.ascontiguousarray(key[b].T).astype(f16),
            "xv": np.ascontiguousarray(value[b].T).astype(f16),
            "wq": np.ascontiguousarray((0.125 * Wq[sl]).T).astype(f16),
            "wk": np.ascontiguousarray(Wk[sl].T).astype(f16),
            "wv": np.ascontiguousarray(Wv[sl].T).astype(f16),
            "bqk": bqk2,
            "bvb": np.broadcast_to(bv[sl], (128, OG)).copy().astype(np.float32),
            "rel": np.ascontiguousarray(rel[b]).astype(np.float32),
            "tsd": np.ascontiguousarray(timestamp[b]).astype(np.float32),
            "msk": m_u8,
            "mneg": (m_u8.astype(np.float32) * MASKNEG).astype(f16),
            "lam": lam,
            "idn16": idn16,
            "idn32": idn32,
        })
    return in_maps


def kernel(query, key, value, rel, timestamp, l1, l2, mask, Wq, bq, Wk, bk, Wv, bv):
    nc = _get_nc()
    in_maps = _make_in_maps(query, key, value, rel, timestamp, l1, l2, mask,
                            Wq, bq, Wk, bk, Wv, bv)
    res = run_bass_kernel_spmd(nc, in_maps, list(range(8)))
    out = np.empty((B, S, D), np.float32)
    prob = np.empty((B, H, S, S), np.float32)
    for cid in range(8):
        b, g = divmod(cid, 2)
        r = res.results[cid]
        prob[b, g * HG:(g + 1) * HG] = r["prob_o"]
        for h in range(HG):
            out[b, :, g * OG + h * HD:g * OG + (h + 1) * HD] = r["outT_o"][h].T
    return out, prob
